# revision 1
# baseline (speedup 1.0000x reference)
"""BoxFilter (9x9 unnormalized box sum, zero-padded borders) on 8 trn2 cores.

Full input: image [8, 32, 512, 512] f32, batch-sharded: core b handles
image[b]. Per channel slice X [512, 512]:

  pass A (H) on PE: Y[i-block, w] = sum_j Band[j, i] X[j, w] using the three
    Toeplitz blocks of the 9-band matrix (diagonal + two corners) as
    fp32r stationaries -- 10 matmuls per slice, full-bank N=512 accumulation.
  pass B (W) on DVE: one tensor_tensor_scan per h-block over a zero-padded
    row: state = (Y[j+9] + state) - Y[j], whose running sum telescopes to the
    9-tap sliding box (lead pad of 9 zeros makes the telescoped constant 0).

One DMA loads all 4 h-chunks of a slice, one DMA stores all 4 h-blocks.
Inputs are pre-rounded to the fp32r grid (TF32-like, 11-bit mantissa); band
entries are exactly 1.0, so results are near-exact (rel err ~6e-7).
"""

import numpy as np

import concourse.bass as bass
import concourse.mybir as mybir
import concourse.tile as tile
from concourse import bacc, bass_utils

RADIUS = 4
H = W = 512
P = 128  # partitions / chunk size
NCHUNK = H // P  # 4
N_CORES = 8
NCH = 32  # channels per core (batch dim sharded across cores)

# moving-window offsets per chunk; chunk 0 uses the full 512 (start=True pass
# must cover the whole PSUM bank so later windowed accumulates see uniform
# has_written state)
WIN_OFF = [0, 64, 192, 256]
WIN_N = [512, 256, 256, 256]
# column offset of chunk t's slab inside the packed band constant
BAND_COL = [0, 512, 768, 1024]
BAND_TOT = 1280


def round_to_fp32r(a: np.ndarray) -> np.ndarray:
    """Round fp32 to the fp32r grid (8-bit exp, 11-bit mantissa: RNE, low 12
    bits zeroed) so the on-device fp32r interpretation is exact."""
    u = np.ascontiguousarray(a).view(np.uint32)
    lsb = (u >> np.uint32(12)) & np.uint32(1)
    r = (u + np.uint32(0x7FF) + lsb) & np.uint32(0xFFFFF000)
    return r.view(np.float32)


def band_constant() -> np.ndarray:
    """[128, 384] f32: the three Toeplitz blocks of the 9-band matrix —
    diagonal block | lower corner (prev chunk) | upper corner (next chunk)."""
    j = np.arange(P)[:, None]
    i = np.arange(P)[None, :]
    b0 = (np.abs(i - j) <= RADIUS).astype(np.float32)
    bm = (np.abs(128 + i - j) <= RADIUS).astype(np.float32)
    bp = (np.abs(i - j - 128) <= RADIUS).astype(np.float32)
    return np.concatenate([b0, bm, bp], axis=1)


YPW = 9 + W + 9  # scan tile: 9 lead + data + 9 tail zeros
OBW = YPW - 9    # scan output width (521); boxW[w] lands at col w + 4


def _emit_pass(nc, pools, band_r, x_ap, y_ap, nch, in_is_f32r, scale=None):
    """Emit the full boxfilter for one [nch, H, W] tensor pair.

    Pass A (H direction) on PE: Y[i-block, w] = sum_j Band[j, i] X[j, w] with
    the three Toeplitz band blocks (diag + 2 corners) as stationaries —
    10 matmuls per slice.  Pass B (W direction) on DVE: one scan-IIR per
    h-block, state = (Y[j+9] + state) - Y[j] over a zero-padded row, which
    emits the full 9-tap running box in a single instruction.
    """
    f32 = mybir.dt.float32
    f32r = mybir.dt.float32r
    const_pool, x_pool, yp_tiles, o_pool, psA, psB = pools
    for c in range(nch):
        # one DMA for all 4 h-chunks: xbig[p, (t, w)] <- x[c, 128t + p, w]
        xbig = x_pool.tile([P, NCHUNK * W], f32r, tag="x")
        src = x_ap[c]
        if not in_is_f32r:
            src = src.bitcast(f32r)
        nc.sync.dma_start(
            xbig[:].rearrange("p (t w) -> p t w", t=NCHUNK),
            src.rearrange("(t p) w -> p t w", p=P),
        )
        xt = [xbig[:, W * t : W * t + W] for t in range(NCHUNK)]

        # all 4 h-blocks in one 4-bank PSUM tile, evacuated by ONE copy and
        # box-summed by ONE scan over the concatenated padded rows (the
        # telescoped window sum is local, and 18 zeros sit between blocks)
        y_ps = psA.tile([P, NCHUNK * W], f32)
        for d in range(NCHUNK):  # h i-block
            mms = [(0, d)]
            if d >= 1:
                mms.append((1, d - 1))
            if d <= NCHUNK - 2:
                mms.append((2, d + 1))
            for k, (m, t) in enumerate(mms):
                nc.tensor.matmul(
                    y_ps[:, W * d : W * d + W],
                    lhsT=band_r[m],
                    rhs=xt[t],
                    start=(k == 0),
                    stop=(k == len(mms) - 1),
                )
        yp = yp_tiles[0]
        nc.vector.tensor_copy(
            yp[:].rearrange("p (d u) -> p d u", d=NCHUNK)[:, :, 9 : 9 + W],
            y_ps[:].rearrange("p (d u) -> p d u", d=NCHUNK),
        )
        obig = o_pool.tile([P, NCHUNK * YPW], f32, tag="o")
        ow = NCHUNK * YPW - 9
        nc.vector.tensor_tensor_scan(
            obig[:, 0:ow],
            yp[:, 9 : NCHUNK * YPW],
            yp[:, 0:ow],
            0.0,
            mybir.AluOpType.add,
            mybir.AluOpType.subtract,
        )
        if scale is not None:
            nc.vector.tensor_scalar_mul(obig[:, 0:ow], obig[:, 0:ow], scale)
        # one DMA for all 4 h-blocks: y[c, 128d + p, w] <- obig[p, YPW*d + 4 + w]
        nc.sync.dma_start(
            y_ap[c].rearrange("(d p) w -> p d w", p=P),
            obig[:].rearrange("p (d u) -> p d u", d=NCHUNK)[:, :, 4 : 4 + W],
        )


def build_nc(nch: int = NCH, chain: int = 1):
    """chain > 1 repeats the filter through internal DRAM scratch (for
    benchmarking: the K-difference isolates pure device time)."""
    f32 = mybir.dt.float32
    f32r = mybir.dt.float32r
    nc = bacc.Bacc("TRN2", target_bir_lowering=False, debug=False)
    x = nc.dram_tensor("x", [nch, H, W], f32r, kind="ExternalInput").ap()
    band_d = nc.dram_tensor("band", [P, 3 * P], f32r, kind="ExternalInput").ap()
    y = nc.dram_tensor("y", [nch, H, W], f32, kind="ExternalOutput").ap()

    with tile.TileContext(nc) as tc:
        with (
            tc.tile_pool(name="const", bufs=1) as const_pool,
            tc.tile_pool(name="xin", bufs=3) as x_pool,
            tc.tile_pool(name="yt", bufs=2) as yt_pool,
            tc.tile_pool(name="osb", bufs=3) as o_pool,
            tc.tile_pool(name="psA", bufs=2, space="PSUM") as psA,
            tc.tile_pool(name="psB", bufs=2, space="PSUM") as psB,
            tc.tile_pool(name="dram", bufs=2, space="DRAM") as dram_pool,
        ):
            band_sb = const_pool.tile([P, 3 * P], f32r)
            nc.sync.dma_start(band_sb[:], band_d[:])
            band_r = [band_sb[:, P * m : P * m + P] for m in range(3)]
            # one persistent concatenated scan tile, zeroed once: the in-loop
            # copy only writes the data columns, so the pads stay zero forever
            t0 = yt_pool.tile([P, NCHUNK * YPW], f32, tag="yp")
            nc.vector.memset(t0[:], 0.0)
            yp_tiles = [t0]
            pools = (const_pool, x_pool, yp_tiles, o_pool, psA, psB)

            scale = None if chain == 1 else 1.0 / 81.0
            cur = x
            cur_f32r = True
            for it in range(chain):
                last = it == chain - 1
                dst = (
                    y
                    if last
                    else dram_pool.tile([nch, H, W], f32, tag="scratch")
                )
                _emit_pass(nc, pools, band_r, cur, dst, nch, cur_f32r, scale)
                cur = dst
                cur_f32r = False

    nc.compile()
    return nc


def kernel(image) -> np.ndarray:
    image = np.ascontiguousarray(np.asarray(image, dtype=np.float32))
    assert image.shape == (N_CORES, NCH, H, W), image.shape
    image = round_to_fp32r(image)
    nc = build_nc(NCH)
    band = band_constant()
    in_maps = [{"x": image[b], "band": band} for b in range(N_CORES)]
    res = bass_utils.run_bass_kernel_spmd(nc, in_maps, core_ids=list(range(N_CORES)))
    return np.stack([r["y"] for r in res.results], axis=0)


if __name__ == "__main__":
    img = np.random.rand(N_CORES, NCH, H, W).astype(np.float32)
    out = kernel(img)
    print(out.shape, out.dtype)



# revision 2
# speedup vs baseline: 2.3024x; 2.3024x over previous
"""BoxFilter (9x9 unnormalized box sum, zero-padded) on 8 trn2 cores.

Minimal-instruction grouped design for an execution environment where each
instruction has a large fixed cost and big 2D DVE ops amortize best:

Per group of G channels (bf16):
  - G per-channel DMAs into padded scan tile xp. Per-channel slot layout
    (free axis, SEG=530): [zs][t0][t1][t2][t3]; t-seg = 9z|512 data|9z,
    zs = 530 zeros. Pads memset once, never rewritten.
  - ONE tensor_tensor_scan (fp32 state) = 9-tap W-box for every row.
  - H-box via a doubling tree where the partition shift of each level is
    materialized by a DMA pair (engines cannot read partition-offset APs;
    DMA can): main shift SH[0:128-s) <- src[s:128) plus chunk-boundary
    wrap SH[128-s:128, c) <- src[0:s, c+SEG). Then one full 2D
    tensor_tensor add. The zs segments accumulate the image-edge partial
    sums exactly as in the validated numpy mock (mock_v3).
  - Final: P = B3 + shift(+8)(B0); O = shift(-4)(P) via 2 DMAs into V.
  - G per-channel DMAs out of the value columns.
"""

import numpy as np
import ml_dtypes

import concourse.mybir as mybir
import concourse.tile as tile
from concourse import bacc, bass_utils

RADIUS = 4
H = W = 512
P = 128
NCHUNK = 4
N_CORES = 8
NCH = 32

SEG = 530
CH = 5 * SEG
G = 7
LTS = G * CH + SEG + 9   # tree/scan-out tile length (tail zs + slack)
LT = LTS + 9             # xp length

BF16 = mybir.dt.bfloat16
ADD = mybir.AluOpType.add


NSPLIT = 4  # col-chunks per main shift: independent DMAs overlap each other


def _shift_up(nc, dst, src, s):
    """dst[p] = src[p+s] rows-wise with chunk wrap: top s partitions read
    the next segment (c+SEG) of the bottom s partitions. The main shift is
    issued as NSPLIT independent col-chunk DMAs so they run concurrently."""
    step = (LTS + NSPLIT - 1) // NSPLIT
    for j in range(0, LTS, step):
        e = min(j + step, LTS)
        nc.sync.dma_start(dst[0 : P - s, j:e], src[s:P, j:e])
    nc.sync.dma_start(dst[P - s : P, 0 : LTS - SEG], src[0:s, SEG:LTS])


def _emit_pass(nc, tiles, x_ap, y_ap, nch):
    xp, xs, a, b, c = tiles
    for g0 in range(0, nch, G):
        gn = min(G, nch - g0)
        for i in range(gn):
            cb = i * CH + SEG
            nc.sync.dma_start(
                xp[:, cb : cb + 4 * SEG]
                .rearrange("p (t c) -> p t c", t=NCHUNK)[:, :, 9 : 9 + W],
                x_ap[g0 + i].rearrange("(t p) w -> p t w", p=P),
            )
        # W pass: one scan
        nc.vector.tensor_tensor_scan(
            xs[:, 0:LTS], xp[:, 9:LT], xp[:, 0:LTS], 0.0,
            ADD, mybir.AluOpType.subtract,
        )
        # H pass: doubling tree (shifts 1,2,4), then P9 = B4 + sh8(xs)
        src = xs
        for dst, s in ((a, 1), (b, 2), (a, 4)):
            _shift_up(nc, c, src, s)
            nc.vector.tensor_tensor(
                dst[:, 0:LTS], src[:, 0:LTS], c[:, 0:LTS], ADD
            )
            src = dst
        _shift_up(nc, c, xs, 8)
        nc.vector.tensor_tensor(b[:, 0:LTS], a[:, 0:LTS], c[:, 0:LTS], ADD)
        # the -4 centering shift folds into the out-DMAs: output row
        # 128d+p reads P9[p-4] (chunk d; prev seg's top 4 rows for p<4)
        for i in range(gn):
            dview = y_ap[g0 + i].rearrange("(d p) w -> p d w", p=P)
            # vv segs: j=0 is the slot zs, j=1..4 are chunks 0..3
            vv = b[:, i * CH : i * CH + 5 * SEG].rearrange(
                "p (j c) -> p j c", j=5
            )
            nc.sync.dma_start(
                dview[4:P], vv[0 : P - 4, 1 : NCHUNK + 1, 4 : 4 + W]
            )
            nc.sync.dma_start(
                dview[0:4], vv[P - 4 : P, 0:NCHUNK, 4 : 4 + W]
            )


def _build(nch: int, chain: int, tiny_io: bool):
    nc = bacc.Bacc("TRN2", target_bir_lowering=False, debug=False)
    xshape = [1, H, W] if tiny_io else [nch, H, W]
    x = nc.dram_tensor("x", xshape, BF16, kind="ExternalInput").ap()
    y = nc.dram_tensor("y", xshape, BF16, kind="ExternalOutput").ap()

    with tile.TileContext(nc) as tc:
        with (
            tc.tile_pool(name="big", bufs=1) as pool,
            tc.tile_pool(name="dram", bufs=2, space="DRAM") as dram_pool,
        ):
            xp = pool.tile([P, LT], BF16, tag="xp")
            xs = pool.tile([P, LTS], BF16, tag="xs")
            a = pool.tile([P, LTS], BF16, tag="a")
            b = pool.tile([P, LTS], BF16, tag="b")
            c = pool.tile([P, LTS], BF16, tag="c")
            nc.vector.memset(xp[:], 0.0)
            nc.vector.memset(xs[:], 0.0)
            nc.vector.memset(a[:], 0.0)
            nc.vector.memset(b[:], 0.0)
            nc.vector.memset(c[:], 0.0)
            tiles = (xp, xs, a, b, c)

            if tiny_io:
                cur = _Bcast(x)
                for it in range(chain):
                    dst = dram_pool.tile([nch, H, W], BF16, tag="scr")
                    _emit_pass(nc, tiles, cur, dst, nch)
                    cur = dst
                nc.sync.dma_start(y[0], cur[0])
            else:
                cur = x
                for it in range(chain):
                    last = it == chain - 1
                    dst = (
                        y if last
                        else dram_pool.tile([nch, H, W], BF16, tag="scr")
                    )
                    _emit_pass(nc, tiles, cur, dst, nch)
                    cur = dst

    nc.compile()
    return nc


class _Bcast:
    def __init__(self, ap):
        self._ap = ap

    def __getitem__(self, c):
        return self._ap[0]


def build_nc(nch: int = NCH, chain: int = 1):
    return _build(nch, chain, tiny_io=False)


def build_bench(k: int, nch: int = NCH):
    return _build(nch, k, tiny_io=True)


def kernel(image) -> np.ndarray:
    image = np.asarray(image)
    assert image.shape == (N_CORES, NCH, H, W), image.shape
    image_bf = np.ascontiguousarray(image).astype(ml_dtypes.bfloat16)
    nc = build_nc(NCH)
    in_maps = [{"x": image_bf[b]} for b in range(N_CORES)]
    res = bass_utils.run_bass_kernel_spmd(nc, in_maps, core_ids=list(range(N_CORES)))
    return np.stack([r["y"].astype(np.float32) for r in res.results], axis=0)


if __name__ == "__main__":
    img = np.random.rand(N_CORES, NCH, H, W).astype(np.float32)
    out = kernel(img)
    print(out.shape, out.dtype)


# revision 3
# speedup vs baseline: 3.6113x; 1.5685x over previous
"""BoxFilter (9x9 unnormalized box sum, zero-padded) on 8 trn2 cores.

Minimal-instruction grouped design for an execution environment where each
instruction has a large fixed cost and big 2D DVE ops amortize best:

Per group of G channels (bf16):
  - G per-channel DMAs into padded scan tile xp. Per-channel slot layout
    (free axis, SEG=530): [zs][t0][t1][t2][t3]; t-seg = 9z|512 data|9z,
    zs = 530 zeros. Pads memset once, never rewritten.
  - ONE tensor_tensor_scan (fp32 state) = 9-tap W-box for every row.
  - H-box via a doubling tree where the partition shift of each level is
    materialized by a DMA pair (engines cannot read partition-offset APs;
    DMA can): main shift SH[0:128-s) <- src[s:128) plus chunk-boundary
    wrap SH[128-s:128, c) <- src[0:s, c+SEG). Then one full 2D
    tensor_tensor add. The zs segments accumulate the image-edge partial
    sums exactly as in the validated numpy mock (mock_v3).
  - Final: P = B3 + shift(+8)(B0); O = shift(-4)(P) via 2 DMAs into V.
  - G per-channel DMAs out of the value columns.
"""

import numpy as np
import ml_dtypes

import concourse.mybir as mybir
import concourse.tile as tile
from concourse import bacc, bass_utils

RADIUS = 4
H = W = 512
P = 128
NCHUNK = 4
N_CORES = 8
NCH = 32

SEG = 530
CH = 5 * SEG
G = 7
LTS = G * CH + SEG + 9   # tree/scan-out tile length (tail zs + slack)
LT = LTS + 9             # xp length

BF16 = mybir.dt.bfloat16
ADD = mybir.AluOpType.add


NSPLIT = 1  # col-chunks per main shift (1: split adds only dispatch cost)


def _shift_up(nc, dst, src, s):
    """dst[p] = src[p+s] rows-wise with chunk wrap: top s partitions read
    the next segment (c+SEG) of the bottom s partitions. The main shift is
    issued as NSPLIT independent col-chunk DMAs so they run concurrently."""
    step = (LTS + NSPLIT - 1) // NSPLIT
    for j in range(0, LTS, step):
        e = min(j + step, LTS)
        nc.sync.dma_start(dst[0 : P - s, j:e], src[s:P, j:e])
    nc.sync.dma_start(dst[P - s : P, 0 : LTS - SEG], src[0:s, SEG:LTS])


def _emit_pass(nc, tiles, x_ap, y_ap, nch):
    xp, xs, a, b, c = tiles
    for g0 in range(0, nch, G):
        gn = min(G, nch - g0)
        for i in range(gn):
            cb = i * CH + SEG
            nc.sync.dma_start(
                xp[:, cb : cb + 4 * SEG]
                .rearrange("p (t c) -> p t c", t=NCHUNK)[:, :, 9 : 9 + W],
                x_ap[g0 + i].rearrange("(t p) w -> p t w", p=P),
            )
        # W pass: one scan
        nc.vector.tensor_tensor_scan(
            xs[:, 0:LTS], xp[:, 9:LT], xp[:, 0:LTS], 0.0,
            ADD, mybir.AluOpType.subtract,
        )
        # H pass: doubling tree (shifts 1,2,4), then P9 = B4 + sh8(xs)
        src = xs
        for dst, s in ((a, 1), (b, 2), (a, 4)):
            _shift_up(nc, c, src, s)
            nc.vector.tensor_tensor(
                dst[:, 0:LTS], src[:, 0:LTS], c[:, 0:LTS], ADD
            )
            src = dst
        _shift_up(nc, c, xs, 8)
        nc.vector.tensor_tensor(b[:, 0:LTS], a[:, 0:LTS], c[:, 0:LTS], ADD)
        # the -4 centering shift folds into the out-DMAs: output row
        # 128d+p reads P9[p-4] (chunk d; prev seg's top 4 rows for p<4)
        for i in range(gn):
            dview = y_ap[g0 + i].rearrange("(d p) w -> p d w", p=P)
            # vv segs: j=0 is the slot zs, j=1..4 are chunks 0..3
            vv = b[:, i * CH : i * CH + 5 * SEG].rearrange(
                "p (j c) -> p j c", j=5
            )
            nc.sync.dma_start(
                dview[4:P], vv[0 : P - 4, 1 : NCHUNK + 1, 4 : 4 + W]
            )
            nc.sync.dma_start(
                dview[0:4], vv[P - 4 : P, 0:NCHUNK, 4 : 4 + W]
            )


def _build(nch: int, chain: int, tiny_io: bool):
    nc = bacc.Bacc("TRN2", target_bir_lowering=False, debug=False)
    xshape = [1, H, W] if tiny_io else [nch, H, W]
    x = nc.dram_tensor("x", xshape, BF16, kind="ExternalInput").ap()
    y = nc.dram_tensor("y", xshape, BF16, kind="ExternalOutput").ap()

    with tile.TileContext(nc) as tc:
        with (
            tc.tile_pool(name="big", bufs=1) as pool,
            tc.tile_pool(name="dram", bufs=2, space="DRAM") as dram_pool,
        ):
            xp = pool.tile([P, LT], BF16, tag="xp")
            xs = pool.tile([P, LTS], BF16, tag="xs")
            a = pool.tile([P, LTS], BF16, tag="a")
            b = pool.tile([P, LTS], BF16, tag="b")
            c = pool.tile([P, LTS], BF16, tag="c")
            nc.vector.memset(xp[:], 0.0)
            nc.vector.memset(xs[:], 0.0)
            nc.vector.memset(a[:], 0.0)
            nc.vector.memset(b[:], 0.0)
            nc.vector.memset(c[:], 0.0)
            tiles = (xp, xs, a, b, c)

            if tiny_io:
                cur = _Bcast(x)
                for it in range(chain):
                    dst = dram_pool.tile([nch, H, W], BF16, tag="scr")
                    _emit_pass(nc, tiles, cur, dst, nch)
                    cur = dst
                nc.sync.dma_start(y[0], cur[0])
            else:
                cur = x
                for it in range(chain):
                    last = it == chain - 1
                    dst = (
                        y if last
                        else dram_pool.tile([nch, H, W], BF16, tag="scr")
                    )
                    _emit_pass(nc, tiles, cur, dst, nch)
                    cur = dst

    nc.compile()
    return nc


class _Bcast:
    def __init__(self, ap):
        self._ap = ap

    def __getitem__(self, c):
        return self._ap[0]


def build_nc(nch: int = NCH, chain: int = 1):
    return _build(nch, chain, tiny_io=False)


def build_bench(k: int, nch: int = NCH):
    return _build(nch, k, tiny_io=True)


def kernel(image) -> np.ndarray:
    image = np.asarray(image)
    assert image.shape == (N_CORES, NCH, H, W), image.shape
    image_bf = np.ascontiguousarray(image).astype(ml_dtypes.bfloat16)
    nc = build_nc(NCH)
    in_maps = [{"x": image_bf[b]} for b in range(N_CORES)]
    res = bass_utils.run_bass_kernel_spmd(nc, in_maps, core_ids=list(range(N_CORES)))
    return np.stack([r["y"].astype(np.float32) for r in res.results], axis=0)


if __name__ == "__main__":
    img = np.random.rand(N_CORES, NCH, H, W).astype(np.float32)
    out = kernel(img)
    print(out.shape, out.dtype)
